# revision 7
# baseline (speedup 1.0000x reference)
"""Trainium2 Bass kernel for nn_Cross_Attention — single-read resident-fp16 design.

Per core (2 samples):
  Loads: 8 SWDGE cast-DMAs x f32->fp16, contiguous, into resident SBUF
         (c-layout [c, h, w]).  Pools via packed fp16 TensorTensor
         fold-trees (4x DVE mode) + short reduces; x_hw needs the
         cross-partition max: c-tile fold (Pool engine) + PE transposes +
         PSUM reduce.  Per-sample attention (fp16 matmuls, f32 softmax)
         emitted right after that sample's pools so sample 1's loads and
         pools overlap sample 0's attention.
  BN:    partial sums -> AllReduce [128,20] -> finalize -> sigmoid gates.
  Apply: out = x*(1 + A*B*C) + x.  Per 16-row slab: PE rank-1
         psum[c,w] = B[:,h] (x) A[h,:] (strips staged via DRAM), Act
         copies psum->fp16 out tile, DVE *C (mid-dim broadcast) then
         (g+1)*x in place (split DVE/Pool), SWDGE cast store fp16->f32.
"""

import os

import numpy as np

import concourse.bacc as bacc
import concourse.mybir as mybir
import concourse.tile as tile
from concourse import masks

f32 = mybir.dt.float32
f16 = mybir.dt.float16
Alu = mybir.AluOpType
Act = mybir.ActivationFunctionType
X = mybir.AxisListType.X

NCORES = 8
S = 2          # samples per core
C, H, W = 256, 128, 128
CT = 2         # c tiles of 128
HC = 16        # h rows per apply slab
GH = 64        # h rows per load half-tile
NG = H // GH   # 2
QH = 32        # h rows per output tile
NP = 20        # bn-partial columns
EPS = 1e-5
PHASES = os.environ.get("KPHASES", "full")  # load|pools|mid|full


def build_bass(n_cores: int):
    nc = bacc.Bacc("TRN2", target_bir_lowering=False, debug=False,
                   num_devices=n_cores)
    nb_tot = n_cores * S
    n1 = float(nb_tot * H * W)
    ncn = float(nb_tot * H)

    xs = nc.dram_tensor("xs", [S, C, H, W], f32, kind="ExternalInput").ap()
    bn1w = nc.dram_tensor("bn1_w", [1], f32, kind="ExternalInput").ap()
    bn1b = nc.dram_tensor("bn1_b", [1], f32, kind="ExternalInput").ap()
    bncw = nc.dram_tensor("bnc_w", [C], f32, kind="ExternalInput").ap()
    bncb = nc.dram_tensor("bnc_b", [C], f32, kind="ExternalInput").ap()
    outy = nc.dram_tensor("outy", [S, C, H, W], f32, kind="ExternalOutput").ap()

    agd = nc.dram_tensor("agd", [S, H, W], f16).ap()
    bgd = nc.dram_tensor("bgd", [S, H, CT * 128], f16).ap()
    ccin = nc.dram_tensor("ccin", [128, NP], f32).ap()
    ccout = nc.dram_tensor(
        "ccout", [128, NP], f32,
        addr_space="Shared" if n_cores > 1 else "Local").ap()

    reps = int(os.environ.get("KREPS", "1"))
    with tile.TileContext(nc) as tc:
        for _ in range(reps):
            _emit(nc, tc, n_cores, n1, ncn,
                  xs, bn1w, bn1b, bncw, bncb, outy, ccin, ccout, agd, bgd)
    nc.compile()
    return nc


def _emit(nc, tc, n_cores, n1, ncn,
          xs, bn1w, bn1b, bncw, bncb, outy, ccin, ccout, agd, bgd):
    import contextlib
    stack = contextlib.ExitStack()
    with stack:
        persist = stack.enter_context(tc.tile_pool(name="persist", bufs=1))
        xres = stack.enter_context(tc.tile_pool(name="xres", bufs=1))
        maps = stack.enter_context(tc.tile_pool(name="maps", bufs=2))
        cols = stack.enter_context(tc.tile_pool(name="cols", bufs=4))
        keep = stack.enter_context(tc.tile_pool(name="keep", bufs=1))
        gscr = stack.enter_context(tc.tile_pool(name="gscr", bufs=4))

        # --- persistent setup ---
        identity = persist.tile([128, 128], f32)
        masks.make_identity(nc, identity[:])
        identity16 = persist.tile([128, 128], f16)
        nc.vector.tensor_copy(identity16[:], identity[:])
        ones_r = persist.tile([1, 128], f32)
        nc.vector.memset(ones_r[:], 1.0)
        ones_c = persist.tile([128, 1], f32)
        nc.vector.memset(ones_c[:], 1.0)
        eps_col = persist.tile([128, 1], f32)
        nc.vector.memset(eps_col[:], EPS)
        wc2 = persist.tile([128, 2], f32)
        nc.sync.dma_start(wc2[:], bncw.rearrange("(t c) -> c t", c=128))
        bc2 = persist.tile([128, 2], f32)
        nc.sync.dma_start(bc2[:], bncb.rearrange("(t c) -> c t", c=128))
        bn1w_sb = persist.tile([1, 1], f32)
        nc.sync.dma_start(bn1w_sb[:], bn1w.unsqueeze(1))
        bn1b_sb = persist.tile([1, 1], f32)
        nc.sync.dma_start(bn1b_sb[:], bn1b.unsqueeze(1))
        wc8 = persist.tile([128, 8], f32)
        bc8 = persist.tile([128, 8], f32)
        for m in range(4):
            nc.vector.tensor_copy(wc8[:, m * 2:m * 2 + 2], wc2[:])
            nc.vector.tensor_copy(bc8[:, m * 2:m * 2 + 2], bc2[:])
        bnp = persist.tile([128, NP], f32)
        nc.vector.memset(bnp[:], 0.0)

        # resident fp16 x: 8 half-tiles [c=128, GH, W] keyed (s, t, g)
        x16 = {}
        for s in range(S):
            for t in range(CT):
                for g in range(NG):
                    x16[(s, t, g)] = xres.tile(
                        [128, GH, W], f16, name=f"x16_{s}{t}{g}",
                        tag=f"x16_{s}{t}{g}")

        def smap(name, shape, dtype=f16, bufs=None):
            return [maps.tile(shape, dtype, name=f"{name}{s}", tag=name,
                              bufs=bufs)
                    for s in range(S)]

        xch = smap("xch", [128, CT * H])     # [c_loc, (t,h)]
        xcw = smap("xcw", [128, CT * W])     # [c_loc, (t,w)]
        xhwT = smap("xhwT", [128, H])        # [w, h]

        scht = smap("scht", [128, CT * H])   # [h, (t,c)]
        scwt = smap("scwt", [128, CT * W])   # [w, (t,c)]
        shw = smap("shw", [128, W])          # [h, w]
        e12 = smap("e12", [128, C], bufs=1)
        e12t = smap("e12t", [128, CT * W], bufs=1)
        e13 = smap("e13", [128, C], bufs=1)
        e13t = smap("e13t", [128, CT * H], bufs=1)
        e23 = smap("e23", [128, W], bufs=1)
        e23t = smap("e23t", [128, H], bufs=1)
        y12T = smap("y12T", [128, H], f32, bufs=1)   # [w, h]
        y12 = smap("y12", [128, W], f32)
        y13 = smap("y13", [128, W], f32)
        y21 = smap("y21", [128, CT * H])             # fp16 [c,(t,h)]
        y23 = smap("y23", [128, CT * H])
        y31 = smap("y31", [128, CT * W])
        y32 = smap("y32", [128, CT * W])
        agate = smap("agate", [128, W])              # [h, w] fp16
        cgate = smap("cgate", [128, CT * W])         # [c, (t,w)] fp16
        bgateT = smap("bgateT", [128, CT * 128])     # [h, (t,c)] fp16

        itc = {}
        it1 = {}

        mid = contextlib.ExitStack()
        with mid:
            pmf = mid.enter_context(tc.tile_pool(name="pmf", bufs=2))
            pcw = mid.enter_context(tc.tile_pool(name="pcw", bufs=2))
            pft = mid.enter_context(tc.tile_pool(name="pft", bufs=1))
            pft2 = mid.enter_context(tc.tile_pool(name="pft2", bufs=1))
            ps_t = mid.enter_context(
                tc.tile_pool(name="ps_t", bufs=2, space="PSUM"))
            ps_mm = mid.enter_context(
                tc.tile_pool(name="ps_mm", bufs=2, space="PSUM"))
            ps_ty = mid.enter_context(
                tc.tile_pool(name="ps_ty", bufs=2, space="PSUM"))

            # ---- all cast-loads up-front on the SWDGE queue ----
            # quarter-granularity (1 MB source) for peak cast-DMA rate and
            # finer load->pool pipelining
            for s in range(S):
                for g in range(NG):
                    for t in range(CT):
                        for j in range(4):
                            nc.gpsimd.dma_start(
                                x16[(s, t, g)][:, j * HC:(j + 1) * HC, :],
                                xs[s, t * 128:(t + 1) * 128,
                                   g * GH + j * HC:g * GH + (j + 1) * HC, :])

            def pools(s):
                xcwacc = {}
                for g in range(NG):
                    for t in range(CT):
                        xh = x16[(s, t, g)]
                        # xch (fold w 128->4) and xcw (fold h 64->1) as
                        # packed TT fold-trees, interleaved so consecutive
                        # DVE ops are independent
                        f2 = pft2.tile([128, GH, 64], f16,
                                       name=f"f2_{s}{t}{g}", tag="f2")
                        ft = pft.tile([128, 32, W], f16,
                                      name=f"ft{s}{t}{g}", tag="ft")
                        nc.vector.tensor_tensor(
                            out=f2[:], in0=xh[:, :, 0:64],
                            in1=xh[:, :, 64:128], op=Alu.max)
                        nc.vector.tensor_tensor(
                            out=ft[:], in0=xh[:, 0:32, :],
                            in1=xh[:, 32:64, :], op=Alu.max)
                        for wd, hd in ((32, 16), (16, 8), (8, 4), (4, 2)):
                            nc.vector.tensor_tensor(
                                out=f2[:, :, 0:wd], in0=f2[:, :, 0:wd],
                                in1=f2[:, :, wd:2 * wd], op=Alu.max)
                            nc.vector.tensor_tensor(
                                out=ft[:, 0:hd, :], in0=ft[:, 0:hd, :],
                                in1=ft[:, hd:2 * hd, :], op=Alu.max)
                        nc.vector.tensor_tensor(
                            out=ft[:, 0:1, :], in0=ft[:, 0:1, :],
                            in1=ft[:, 1:2, :], op=Alu.max)
                        nc.vector.tensor_reduce(
                            out=xch[s][:, t * H + g * GH:
                                       t * H + (g + 1) * GH],
                            in_=f2[:, :, 0:4], axis=X, op=Alu.max)
                        if g == 0:
                            cw = pcw.tile([128, W], f16,
                                          name=f"cw{s}{t}", tag="cw")
                            nc.vector.tensor_copy(cw[:], ft[:, 0, :])
                            xcwacc[t] = cw
                        else:
                            nc.vector.tensor_tensor(
                                out=xcw[s][:, t * W:(t + 1) * W],
                                in0=xcwacc[t][:], in1=ft[:, 0, :],
                                op=Alu.max)
                    # x_hw: c-tile fold + PE transpose + PSUM reduce
                    for j in range(4):
                        k = g * 4 + j
                        mf = pmf.tile([128, HC, W], f16, name=f"mf{s}{k}",
                                      tag="mf")
                        nc.vector.tensor_tensor(
                            out=mf[:],
                            in0=x16[(s, 0, g)][:, j * HC:(j + 1) * HC, :],
                            in1=x16[(s, 1, g)][:, j * HC:(j + 1) * HC, :],
                            op=Alu.max)
                        tr = ps_t.tile([128, HC, 128], f16,
                                       name=f"tr{s}{k}", tag="tr")
                        for jj in range(HC):
                            nc.tensor.transpose(
                                tr[:, jj, :], mf[:, jj, :], identity16[:])
                        nc.vector.tensor_reduce(
                            out=xhwT[s][:, k * HC:(k + 1) * HC],
                            in_=tr[:], axis=X, op=Alu.max)

            def psum_copy_to(dst, src_ps):
                nc.scalar.copy(dst, src_ps)

            def transpose16_to(dst, src_sb, nblk, name):
                for t in range(nblk):
                    tp = ps_mm.tile([128, 128], f16, name=f"tp{name}{t}",
                                    tag="mm")
                    nc.tensor.transpose(
                        tp[:], src_sb[:, t * 128:(t + 1) * 128],
                        identity16[:])
                    psum_copy_to(dst[:, t * 128:(t + 1) * 128], tp[:])

            def softmax(s, br, sim_ps, e_dst):
                rowmax = cols.tile([128, 1], f32, name=f"rm{s}{br}", tag="c1")
                nc.vector.tensor_reduce(out=rowmax[:], in_=sim_ps[:], axis=X,
                                        op=Alu.max)
                rmt = ps_ty.tile([1, 128], f32, name=f"rmt{s}{br}", tag="ty")
                nc.tensor.transpose(rmt[:], rowmax[:], identity[:])
                gmax = cols.tile([1, 1], f32, name=f"gm{s}{br}", tag="c0")
                nc.vector.tensor_reduce(out=gmax[:], in_=rmt[:], axis=X,
                                        op=Alu.max)
                ngmax = cols.tile([1, 1], f32, name=f"ngm{s}{br}", tag="c0")
                nc.scalar.mul(ngmax[:], gmax[:], -1.0)
                nm_ps = ps_ty.tile([128, 1], f32, name=f"nmp{s}{br}", tag="ty")
                nc.tensor.matmul(nm_ps[:], ones_r[:], ngmax[:])
                nmcol = cols.tile([128, 1], f32, name=f"nmc{s}{br}", tag="c1")
                psum_copy_to(nmcol[:], nm_ps[:])
                rowsum = cols.tile([128, 1], f32, name=f"rs{s}{br}", tag="c1")
                nc.scalar.activation(out=e_dst[:], in_=sim_ps[:], func=Act.Exp,
                                     bias=nmcol[:], scale=1.0,
                                     accum_out=rowsum[:])
                tot_ps = ps_ty.tile([1, 1], f32, name=f"tot{s}{br}", tag="ty")
                nc.tensor.matmul(tot_ps[:], rowsum[:], ones_c[:])
                invt = keep.tile([1, 1], f32, name=f"it{s}{br}",
                                 tag=f"it{s}{br}")
                nc.vector.reciprocal(invt[:], tot_ps[:])
                ic_ps = ps_ty.tile([128, 1], f32, name=f"icp{s}{br}", tag="ty")
                nc.tensor.matmul(ic_ps[:], ones_r[:], invt[:])
                iccol = keep.tile([128, 1], f32, name=f"icc{s}{br}",
                                  tag=f"icc{s}{br}")
                psum_copy_to(iccol[:], ic_ps[:])
                it1[(s, br)] = invt
                itc[(s, br)] = iccol

            def attn(s):
                transpose16_to(scht[s], xch[s], CT, f"sch{s}")
                transpose16_to(scwt[s], xcw[s], CT, f"scw{s}")
                shp = ps_mm.tile([128, 128], f16, name=f"shp{s}", tag="mm")
                nc.tensor.transpose(shp[:], xhwT[s][:], identity16[:])
                psum_copy_to(shw[s][:], shp[:])

                sim12 = ps_mm.tile([128, C], f32, name=f"s12_{s}", tag="mm")
                nc.tensor.matmul(sim12[:], shw[s][:], scht[s][:])
                softmax(s, 12, sim12, e12[s])
                transpose16_to(e12t[s], e12[s], CT, f"e12{s}")
                y12p = ps_mm.tile([128, H], f32, name=f"y12p{s}", tag="mm")
                for t in range(CT):
                    nc.tensor.matmul(
                        y12p[:], e12t[s][:, t * W:(t + 1) * W],
                        xch[s][:, t * H:(t + 1) * H],
                        start=(t == 0), stop=(t == CT - 1))
                psum_copy_to(y12T[s][:], y12p[:])
                for t in range(CT):
                    y21p = ps_mm.tile([128, H], f32, name=f"y21p{s}{t}",
                                      tag="mm")
                    nc.tensor.matmul(y21p[:], e12[s][:, t * 128:(t + 1) * 128],
                                     xhwT[s][:])
                    psum_copy_to(y21[s][:, t * H:(t + 1) * H], y21p[:])

                sim13 = ps_mm.tile([128, C], f32, name=f"s13_{s}", tag="mm")
                nc.tensor.matmul(sim13[:], xhwT[s][:], scwt[s][:])
                softmax(s, 13, sim13, e13[s])
                transpose16_to(e13t[s], e13[s], CT, f"e13{s}")
                y13p = ps_mm.tile([128, W], f32, name=f"y13p{s}", tag="mm")
                for t in range(CT):
                    nc.tensor.matmul(
                        y13p[:], e13t[s][:, t * H:(t + 1) * H],
                        xcw[s][:, t * W:(t + 1) * W],
                        start=(t == 0), stop=(t == CT - 1))
                psum_copy_to(y13[s][:], y13p[:])
                for t in range(CT):
                    y31p = ps_mm.tile([128, W], f32, name=f"y31p{s}{t}",
                                      tag="mm")
                    nc.tensor.matmul(y31p[:], e13[s][:, t * 128:(t + 1) * 128],
                                     shw[s][:])
                    psum_copy_to(y31[s][:, t * W:(t + 1) * W], y31p[:])

                sim23 = ps_mm.tile([128, W], f32, name=f"s23_{s}", tag="mm")
                for t in range(CT):
                    nc.tensor.matmul(
                        sim23[:], xch[s][:, t * H:(t + 1) * H],
                        xcw[s][:, t * W:(t + 1) * W],
                        start=(t == 0), stop=(t == CT - 1))
                softmax(s, 23, sim23, e23[s])
                transpose16_to(e23t[s], e23[s], 1, f"e23{s}")
                for t in range(CT):
                    y23p = ps_mm.tile([128, H], f32, name=f"y23p{s}{t}",
                                      tag="mm")
                    nc.tensor.matmul(y23p[:], scwt[s][:, t * W:(t + 1) * W],
                                     e23t[s][:])
                    psum_copy_to(y23[s][:, t * H:(t + 1) * H], y23p[:])
                    y32p = ps_mm.tile([128, W], f32, name=f"y32p{s}{t}",
                                      tag="mm")
                    nc.tensor.matmul(y32p[:], scht[s][:, t * H:(t + 1) * H],
                                     e23[s][:])
                    psum_copy_to(y32[s][:, t * W:(t + 1) * W], y32p[:])

                y12pp = ps_mm.tile([128, 128], f32, name=f"y12pp{s}", tag="mm")
                nc.tensor.transpose(y12pp[:], y12T[s][:], identity[:])
                psum_copy_to(y12[s][:], y12pp[:])

            ysq = gscr.tile([128, 128], f32, name="ysq", tag="ysq", bufs=2)

            def bnpart(s):
                it2 = {}
                for br in (12, 13, 23):
                    t2 = keep.tile([128, 1], f32, name=f"it2_{s}{br}",
                                   tag=f"it2_{s}{br}")
                    nc.vector.tensor_tensor(out=t2[:], in0=itc[(s, br)][:],
                                            in1=itc[(s, br)][:], op=Alu.mult)
                    it2[br] = t2
                bnc_maps = [(0, y21[s], 12), (1, y23[s], 23),
                            (2, y31[s], 13), (3, y32[s], 23)]
                for m, ysb, br in bnc_maps:
                    r2 = cols.tile([128, 2], f32, name=f"r{s}{m}", tag="c2")
                    nc.vector.tensor_reduce(
                        out=r2[:], in_=ysb[:].rearrange("p (t h) -> p t h",
                                                        t=CT),
                        axis=X, op=Alu.add)
                    nc.vector.scalar_tensor_tensor(
                        out=bnp[:, m * 2:m * 2 + 2], in0=r2[:],
                        scalar=itc[(s, br)][:], in1=bnp[:, m * 2:m * 2 + 2],
                        op0=Alu.mult, op1=Alu.add)
                    for t in range(CT):
                        col = m * 2 + t
                        blk = ysb[:, t * 128:(t + 1) * 128]
                        sq = cols.tile([128, 1], f32, name=f"sq{s}{m}{t}",
                                       tag="c1")
                        nc.scalar.activation(out=ysq[:], in_=blk,
                                             func=Act.Square, accum_out=sq[:])
                        nc.vector.scalar_tensor_tensor(
                            out=bnp[:, 8 + col:9 + col], in0=sq[:],
                            scalar=it2[br][:], in1=bnp[:, 8 + col:9 + col],
                            op0=Alu.mult, op1=Alu.add)
                for j, (ymap, br) in enumerate(((y12T[s], 12), (y13[s], 13))):
                    i1 = it1[(s, br)]
                    i2 = cols.tile([1, 1], f32, name=f"i2_{s}{j}", tag="c0")
                    nc.vector.tensor_tensor(out=i2[:], in0=i1[:], in1=i1[:],
                                            op=Alu.mult)
                    rs = cols.tile([128, 1], f32, name=f"rs1_{s}{j}", tag="c1")
                    nc.vector.tensor_reduce(out=rs[:], in_=ymap[:], axis=X,
                                            op=Alu.add)
                    tp = ps_ty.tile([1, 1], f32, name=f"t1_{s}{j}", tag="ty")
                    nc.tensor.matmul(tp[:], rs[:], ones_c[:])
                    nc.vector.scalar_tensor_tensor(
                        out=bnp[0:1, 16 + 2 * j:17 + 2 * j], in0=tp[:],
                        scalar=i1[:], in1=bnp[0:1, 16 + 2 * j:17 + 2 * j],
                        op0=Alu.mult, op1=Alu.add)
                    sqc = cols.tile([128, 1], f32, name=f"sqc{s}{j}", tag="c1")
                    nc.scalar.activation(out=ysq[:], in_=ymap[:],
                                         func=Act.Square, accum_out=sqc[:])
                    tp2 = ps_ty.tile([1, 1], f32, name=f"t2_{s}{j}", tag="ty")
                    nc.tensor.matmul(tp2[:], sqc[:], ones_c[:])
                    nc.vector.scalar_tensor_tensor(
                        out=bnp[0:1, 17 + 2 * j:18 + 2 * j], in0=tp2[:],
                        scalar=i2[:], in1=bnp[0:1, 17 + 2 * j:18 + 2 * j],
                        op0=Alu.mult, op1=Alu.add)

            if PHASES != "load":
                for s in range(S):
                    pools(s)
                    if PHASES != "pools":
                        attn(s)
                        bnpart(s)
            if PHASES in ("load", "pools"):
                fin = persist.tile([128, 1], f32, name="fin")
                nc.vector.memset(fin[:], 1.0)
                nc.sync.dma_start(outy[0, 0:128, 0, 0:1], fin[:])
                return

            # ---------------- allreduce ----------------
            nc.sync.dma_start(ccin, bnp[:])
            if n_cores > 1:
                nc.gpsimd.collective_compute(
                    "AllReduce", Alu.add,
                    replica_groups=[list(range(n_cores))],
                    ins=[ccin], outs=[ccout])
            else:
                nc.sync.dma_start(ccout, ccin)
            bnpg = persist.tile([128, NP], f32)
            nc.sync.dma_start(bnpg[:], ccout)

            # ---------------- BN finalize + gates ----------------
            sm = persist.tile([128, 8], f32, name="mu8")
            nc.vector.tensor_scalar_mul(sm[:], bnpg[:, 0:8], 1.0 / ncn)
            m2 = persist.tile([128, 8], f32, name="m28")
            nc.vector.tensor_tensor(out=m2[:], in0=sm[:], in1=sm[:],
                                    op=Alu.mult)
            var8 = persist.tile([128, 8], f32, name="var8")
            nc.vector.scalar_tensor_tensor(
                out=var8[:], in0=bnpg[:, 8:16], scalar=1.0 / ncn, in1=m2[:],
                op0=Alu.mult, op1=Alu.subtract)
            sd8 = persist.tile([128, 8], f32, name="sd8")
            nc.scalar.activation(out=sd8[:], in_=var8[:], func=Act.Sqrt,
                                 bias=eps_col[:])
            rstd8 = persist.tile([128, 8], f32, name="rstd8")
            nc.vector.reciprocal(rstd8[:], sd8[:])
            scale8 = persist.tile([128, 8], f32, name="scale8")
            nc.vector.tensor_tensor(out=scale8[:], in0=rstd8[:], in1=wc8[:],
                                    op=Alu.mult)
            q8 = persist.tile([128, 8], f32, name="q8")
            nc.vector.tensor_tensor(out=q8[:], in0=sm[:], in1=scale8[:],
                                    op=Alu.mult)
            shift8 = persist.tile([128, 8], f32, name="shift8")
            nc.vector.scalar_tensor_tensor(
                out=shift8[:], in0=q8[:], scalar=-1.0, in1=bc8[:],
                op0=Alu.mult, op1=Alu.add)

            sc1 = []
            sh1col = []
            for j in range(2):
                mu1 = cols.tile([1, 1], f32, name=f"mu1_{j}", tag="c0")
                nc.vector.tensor_scalar_mul(
                    mu1[:], bnpg[0:1, 16 + 2 * j:17 + 2 * j], 1.0 / n1)
                m21 = cols.tile([1, 1], f32, name=f"m21_{j}", tag="c0")
                nc.vector.tensor_tensor(out=m21[:], in0=mu1[:], in1=mu1[:],
                                        op=Alu.mult)
                v1 = cols.tile([1, 1], f32, name=f"v1_{j}", tag="c0")
                nc.vector.scalar_tensor_tensor(
                    out=v1[:], in0=bnpg[0:1, 17 + 2 * j:18 + 2 * j],
                    scalar=1.0 / n1, in1=m21[:], op0=Alu.mult,
                    op1=Alu.subtract)
                sd1 = cols.tile([1, 1], f32, name=f"sd1_{j}", tag="c0")
                nc.scalar.activation(out=sd1[:], in_=v1[:], func=Act.Sqrt,
                                     bias=eps_col[0:1, :])
                rst1 = cols.tile([1, 1], f32, name=f"rst1_{j}", tag="c0")
                nc.vector.reciprocal(rst1[:], sd1[:])
                sc = keep.tile([1, 1], f32, name=f"sc1_{j}", tag=f"sc1_{j}")
                nc.vector.tensor_tensor(out=sc[:], in0=rst1[:], in1=bn1w_sb[:],
                                        op=Alu.mult)
                sc1.append(sc)
                q1 = cols.tile([1, 1], f32, name=f"q1_{j}", tag="c0")
                nc.vector.tensor_tensor(out=q1[:], in0=mu1[:], in1=sc[:],
                                        op=Alu.mult)
                sh = cols.tile([1, 1], f32, name=f"sh1_{j}", tag="c0")
                nc.vector.scalar_tensor_tensor(
                    out=sh[:], in0=q1[:], scalar=-1.0, in1=bn1b_sb[:],
                    op0=Alu.mult, op1=Alu.add)
                shp_ = ps_ty.tile([128, 1], f32, name=f"shp1_{j}", tag="ty")
                nc.tensor.matmul(shp_[:], ones_r[:], sh[:])
                shcol = keep.tile([128, 1], f32, name=f"shc1_{j}",
                                  tag=f"shc1_{j}")
                psum_copy_to(shcol[:], shp_[:])
                sh1col.append(shcol)

            for s in range(S):
                g1 = gscr.tile([128, W], f32, name=f"g12_{s}", tag="ga")
                g2 = gscr.tile([128, W], f32, name=f"g13_{s}", tag="ga")
                for j, (ymap, br, g) in enumerate(
                        ((y12[s], 12, g1), (y13[s], 13, g2))):
                    scs = cols.tile([1, 1], f32, name=f"scs{s}{j}", tag="c0")
                    nc.vector.tensor_tensor(out=scs[:], in0=sc1[j][:],
                                            in1=it1[(s, br)][:], op=Alu.mult)
                    scp = ps_ty.tile([128, 1], f32, name=f"scp{s}{j}",
                                     tag="ty")
                    nc.tensor.matmul(scp[:], ones_r[:], scs[:])
                    sccol = cols.tile([128, 1], f32, name=f"sccol{s}{j}",
                                      tag="c1")
                    psum_copy_to(sccol[:], scp[:])
                    nc.scalar.activation(out=g[:], in_=ymap[:],
                                         func=Act.Sigmoid, bias=sh1col[j][:],
                                         scale=sccol[:])
                nc.vector.tensor_tensor(out=agate[s][:], in0=g1[:], in1=g2[:],
                                        op=Alu.mult)
                bgate_s = gscr.tile([128, CT * 128], f16, name=f"bg{s}",
                                    tag="gb")
                for gate, (ma, bra), (mb, brb), ysa, ysb_ in (
                        (bgate_s, (0, 12), (1, 23), y21[s], y23[s]),
                        (cgate[s], (2, 13), (3, 23), y31[s], y32[s])):
                    ga = gscr.tile([128, CT * 128], f32, name=f"ga{s}{ma}",
                                   tag="gc")
                    gb = gscr.tile([128, CT * 128], f32, name=f"gb{s}{mb}",
                                   tag="gc")
                    for (m, br, ysrc, gdst) in ((ma, bra, ysa, ga),
                                                (mb, brb, ysb_, gb)):
                        for t in range(CT):
                            col = m * 2 + t
                            scc = cols.tile([128, 1], f32,
                                            name=f"scc{s}{m}{t}", tag="c1")
                            nc.vector.tensor_tensor(
                                out=scc[:], in0=scale8[:, col:col + 1],
                                in1=itc[(s, br)][:], op=Alu.mult)
                            nc.scalar.activation(
                                out=gdst[:, t * 128:(t + 1) * 128],
                                in_=ysrc[:, t * 128:(t + 1) * 128],
                                func=Act.Sigmoid,
                                bias=shift8[:, col:col + 1], scale=scc[:])
                    nc.vector.tensor_tensor(out=gate[:], in0=ga[:], in1=gb[:],
                                            op=Alu.mult)
                for t in range(CT):
                    bp = ps_mm.tile([128, 128], f16, name=f"bgT{s}{t}",
                                    tag="mm")
                    nc.tensor.transpose(
                        bp[:], bgate_s[:, t * 128:(t + 1) * 128],
                        identity16[:])
                    psum_copy_to(bgateT[s][:, t * 128:(t + 1) * 128], bp[:])
                nc.scalar.dma_start(agd[s], agate[s][:])
                nc.scalar.dma_start(bgd[s], bgateT[s][:])

        if PHASES == "mid":
            fin2 = persist.tile([128, 1], f32, name="fin2")
            nc.vector.memset(fin2[:], 1.0)
            nc.sync.dma_start(outy[0, 0:128, 0, 0:1], fin2[:])
            return
        # ---------------- apply ----------------
        p3 = contextlib.ExitStack()
        with p3:
            ps_a = p3.enter_context(
                tc.tile_pool(name="ps_a", bufs=2, space="PSUM"))
            pstr = p3.enter_context(tc.tile_pool(name="pstr", bufs=3))
            pout = p3.enter_context(tc.tile_pool(name="pout", bufs=2))
            for s in range(S):
                for t in range(CT):
                    for q in range(H // QH):
                        o16 = pout.tile([128, QH, W], f16,
                                        name=f"o16_{s}{t}{q}", tag="o16")
                        for j in range(QH // HC):
                            k = q * (QH // HC) + j
                            astr = pstr.tile([1, HC * W], f16,
                                             name=f"as{s}{t}{k}", tag="astr")
                            nc.sync.dma_start(
                                astr[:].rearrange("p (h w) -> p h w", h=HC),
                                agd[s, k * HC:(k + 1) * HC, :].unsqueeze(0))
                            bstr = pstr.tile([1, HC * 128], f16,
                                             name=f"bs{s}{t}{k}", tag="bstr")
                            nc.sync.dma_start(
                                bstr[:].rearrange("p (h c) -> p h c", h=HC),
                                bgd[s, k * HC:(k + 1) * HC,
                                    t * 128:(t + 1) * 128].unsqueeze(0))
                            psa = ps_a.tile([128, HC, W], f32,
                                            name=f"psa{s}{t}{k}", tag="psa")
                            for jj in range(HC):
                                nc.tensor.matmul(
                                    psa[:, jj, :],
                                    bstr[:, jj * 128:(jj + 1) * 128],
                                    astr[:, jj * W:(jj + 1) * W])
                            osl = o16[:, j * HC:(j + 1) * HC, :]
                            nc.scalar.copy(osl, psa[:])
                            nc.vector.tensor_tensor(
                                out=osl, in0=osl,
                                in1=cgate[s][:, t * W:(t + 1) * W]
                                .unsqueeze(1).broadcast_to([128, HC, W]),
                                op=Alu.mult)
                            g = k // 4
                            nc.vector.scalar_tensor_tensor(
                                out=osl, in0=osl, scalar=1.0,
                                in1=x16[(s, t, g)][:, (k % 4) * HC:
                                                   (k % 4 + 1) * HC, :],
                                op0=Alu.add, op1=Alu.mult)
                        nc.gpsimd.dma_start(
                            outy[s, t * 128:(t + 1) * 128,
                                 q * QH:(q + 1) * QH, :],
                            o16[:])


_NC_CACHE = {}
LAST_RESULT = None


def _get_nc(n_cores: int):
    if n_cores not in _NC_CACHE:
        _NC_CACHE[n_cores] = build_bass(n_cores)
    return _NC_CACHE[n_cores]


def make_in_maps(inputs):
    x = np.ascontiguousarray(inputs["x"], dtype=np.float32)
    bn1_w = np.ascontiguousarray(inputs["bn1_w"], dtype=np.float32)
    bn1_b = np.ascontiguousarray(inputs["bn1_b"], dtype=np.float32)
    bnc_w = np.ascontiguousarray(inputs["bnc_w"], dtype=np.float32)
    bnc_b = np.ascontiguousarray(inputs["bnc_b"], dtype=np.float32)
    B = x.shape[0]
    assert B == NCORES * S, (B, NCORES, S)
    in_maps = []
    for i in range(NCORES):
        in_maps.append({
            "xs": np.ascontiguousarray(x[i * S:(i + 1) * S]),
            "bn1_w": bn1_w, "bn1_b": bn1_b,
            "bnc_w": bnc_w, "bnc_b": bnc_b,
        })
    return in_maps


def kernel(**inputs) -> np.ndarray:
    from concourse.bass_utils import run_bass_kernel_spmd

    nc = _get_nc(NCORES)
    in_maps = make_in_maps(inputs)
    res = run_bass_kernel_spmd(nc, in_maps, core_ids=list(range(NCORES)))
    global LAST_RESULT
    LAST_RESULT = res
    out = np.concatenate([res.results[i]["outy"] for i in range(NCORES)],
                         axis=0)
    return out
